# revision 50
# baseline (speedup 1.0000x reference)
"""Multi-head attention (B=8, N=1024, C=768, H=12) on 8 Trainium2 NeuronCores.

Sharding: data-parallel, one batch element per core. Each core computes the
full attention block for its batch: QKV projection, per-head softmax(QK^T/8)V,
and the output projection, entirely on-chip (SBUF/PSUM).

Layout (no on-device transposes):
  - host passes x^T [C, N], w_qkv^T [C, 3C], w_proj^T [C, C] in fp16, bias
    replicated to [128, C] f32.
  - Q, K are produced transposed ([d, n], head-dim on partitions); V in
    natural [n, d] layout by swapping lhsT/rhs.
  - scores are computed transposed (S^T[m, n] = K Q^T) so exp(S^T) feeds the
    P@V matmul as the moving operand; V tiles carry an appended ones-column
    so P@V's 65th output row is the softmax denominator for free.
  - normalization multiplies by a reciprocal row broadcast across partitions
    via a DRAM-bounced DMA (SBUF APs cannot partition-broadcast).

Matmul operands are fp16: the warm PE streams 16-bit moving operands at
1 col/cycle @2.4GHz vs 2 cyc/col for fp32r (HW-measured). PSUM stays f32.
The two K=64 score matmuls per tile run concurrently on PE row groups 0/64.

Schedule (the fp32r version was serialization-bound, not engine-bound):
  - the attention i-loop is software-pipelined: scores for iteration n+1 are
    emitted BEFORE P@V of iteration n, so the in-order PE queue never
    head-of-line-blocks on ScalarE's exp;
  - remaining QKV chains and per-(pair,i) V chains are drip-fed as ~2-matmul
    "filler" steps inside the loop (ScalarE is the bottleneck engine at
    ~1.1us/iter; PE has ~0.35us/iter of slack), instead of block-sized lumps
    that starve ScalarE at pair boundaries;
  - input DMAs are split so the critical slices (x j0-halves, pair-0/6
    weight columns) land first, issued from the Sync + Vector queues in
    parallel, with all bulk (x j1, V/proj weights, remaining QKV columns) on
    the fast-dispatch GPSIMD queue;
  - the softmax-denominator bounce + odd-head repartition DMAs also ride the
    GPSIMD queue;
  - PSUM: psS = 2 x [128,1024] (2 banks each) for the pipelined score tiles,
    psY = 4 x [128,512] (1 bank each): 2 pinned by the P@V accumulators of
    the active pair, 2 rotating among filler/projection chains.
"""

import sys
from collections import deque

import numpy as np

if "/opt/trn_rl_repo" not in sys.path:
    sys.path.insert(0, "/opt/trn_rl_repo")

B = 8
N = 1024
C = 768
H = 12
D = 64
SCALE = D ** -0.5
KT = C // 128           # 6 contraction tiles over channels
NT = N // 128           # 8 token tiles
PAIRS = H // 2          # 6 head pairs

_CACHE = {}


def build_program(**_ignored):
    import concourse.bacc as bacc
    import concourse.mybir as mybir
    import concourse.tile as tile

    f32 = mybir.dt.float32
    f16 = mybir.dt.float16
    Exp = mybir.ActivationFunctionType.Exp

    nc = bacc.Bacc("TRN2", target_bir_lowering=False, debug=False)

    xT_d = nc.dram_tensor("xT", [C, N], f16, kind="ExternalInput")
    wqkvT_d = nc.dram_tensor("wqkvT", [C, 3 * C], f16, kind="ExternalInput")
    wprojT_d = nc.dram_tensor("wprojT", [C, C], f16, kind="ExternalInput")
    bias_d = nc.dram_tensor("bias_rep", [128, C], f32, kind="ExternalInput")
    y_d = nc.dram_tensor("y", [N, C], f32, kind="ExternalOutput")

    mm = nc.tensor.matmul

    with tile.TileContext(nc) as tc:
        with tc.tile_pool(name="pers", bufs=1) as pers, \
             tc.tile_pool(name="cyc", bufs=2) as pB, \
             tc.tile_pool(name="dramb", bufs=2, space="DRAM") as pDr, \
             tc.tile_pool(name="ps_s", bufs=2, space="PSUM") as psS, \
             tc.tile_pool(name="ps_y", bufs=4, space="PSUM") as psY:
            # Q^T,K^T tiles [d, n]: tile m holds heads 2m (parts 0:64) and
            # 2m+1 (parts 64:128); m 0..5 = Q, 6..11 = K.  attn_out^T gets
            # its own tiles (SBUF is not tight in fp16).
            qkt = [pers.tile([128, N], f16, name=f"qkt{m}", tag=f"qkt{m}")
                   for m in range(2 * PAIRS)]
            aot = [pers.tile([128, N], f16, name=f"aot{t}", tag=f"aot{t}")
                   for t in range(PAIRS)]
            # V tiles per (n-tile, pair): [128, 130] = [V_h0 |1| V_h1 |1];
            # ones cols 64/129 feed the denominator row of P@V.  Split per
            # pair so filler writes to pair p never serialize pair p' reads.
            vbuf = [[pers.tile([128, 130], f16, name=f"vb{i}_{p}",
                               tag=f"vb{i}_{p}")
                     for p in range(PAIRS)] for i in range(NT)]
            xt = [pers.tile([128, N], f16, name=f"xt{k}", tag=f"xt{k}")
                  for k in range(KT)]
            wqk = [pers.tile([128, 2 * C], f16, name=f"wqk{k}", tag=f"wqk{k}")
                   for k in range(KT)]
            wv = [pers.tile([128, C], f16, name=f"wv{k}", tag=f"wv{k}")
                  for k in range(KT)]
            wp = [pers.tile([128, C], f16, name=f"wp{k}", tag=f"wp{k}")
                  for k in range(KT)]
            bias_t = pers.tile([128, C], f32, name="bias_t", tag="bias_t")

            # ---- input DMAs: critical slices first, queues in parallel ----
            # sync: x halves (contraction operand of every early chain)
            for k in range(KT):
                nc.sync.dma_start(xt[k][:, 0:512],
                                  xT_d[128 * k:128 * (k + 1), 0:512])
            for k in range(KT):
                nc.sync.dma_start(xt[k][:, 512:1024],
                                  xT_d[128 * k:128 * (k + 1), 512:1024])
            # scalar queue (idle until the first exp): pair-0 V columns
            for k in range(KT):
                nc.scalar.dma_start(wv[k][:, 0:128],
                                    wqkvT_d[128 * k:128 * (k + 1),
                                            2 * C:2 * C + 128])
            # gpsimd: pair-0/6 weight columns first, then all bulk
            for k in range(KT):
                nc.gpsimd.dma_start(wqk[k][:, 0:128],
                                    wqkvT_d[128 * k:128 * (k + 1), 0:128])
            for k in range(KT):
                nc.gpsimd.dma_start(wqk[k][:, 768:896],
                                    wqkvT_d[128 * k:128 * (k + 1), 768:896])
            for k in range(KT):
                nc.gpsimd.dma_start(wv[k][:, 128:768],
                                    wqkvT_d[128 * k:128 * (k + 1),
                                            2 * C + 128:3 * C])
            for k in range(KT):
                nc.gpsimd.dma_start(wqk[k][:, 128:768],
                                    wqkvT_d[128 * k:128 * (k + 1), 128:768])
            for k in range(KT):
                nc.gpsimd.dma_start(wqk[k][:, 896:1536],
                                    wqkvT_d[128 * k:128 * (k + 1), 896:1536])
            for k in range(KT):
                nc.gpsimd.dma_start(wp[k][:],
                                    wprojT_d[128 * k:128 * (k + 1), :])
            nc.gpsimd.dma_start(bias_t[:], bias_d[:])

            for i in range(NT):
                for p in range(PAIRS):
                    ones_ap = vbuf[i][p].rearrange("q (t c) -> q t c",
                                                   c=65)[:, :, 64]
                    nc.vector.memset(ones_ap, 1.0)

            # ---- chain emitters (6-matmul accumulation + DVE finish) ----
            def qk_chain_steps(m, j):
                """Q/K projection chain: qkt[m][:, 512j:512j+512]."""
                ps = psY.tile([128, 512], f32, name="qk_ps", tag="py")
                for k in range(KT):
                    mm(ps[:], wqk[k][:, 128 * m:128 * (m + 1)],
                       xt[k][:, 512 * j:512 * (j + 1)],
                       start=(k == 0), stop=(k == KT - 1))
                    yield 512
                nc.vector.tensor_copy(qkt[m][:, 512 * j:512 * (j + 1)], ps[:])

            def v_chain_steps(p, i):
                """V chain for (pair p, n-tile i) -> vbuf[i][p] data cols."""
                ps = psY.tile([128, 512], f32, name="v_ps", tag="py")
                for k in range(KT):
                    mm(ps[:, 0:128], xt[k][:, 128 * i:128 * (i + 1)],
                       wv[k][:, 128 * p:128 * (p + 1)],
                       start=(k == 0), stop=(k == KT - 1))
                    yield 128
                dst = vbuf[i][p].rearrange("q (t c) -> q t c", c=65)[:, :, 0:64]
                nc.vector.tensor_copy(
                    dst, ps[:, 0:128].rearrange("q (t c) -> q t c", c=64))

            def run_chain(g):
                for _ in g:
                    pass

            # filler queue: drip-feeds chain matmuls into the attention loop.
            # Two chains active at once (two rotating psY slots).  Each chain
            # is tagged with the (pair, j) block whose attention reads its
            # output; before that block's first scores are emitted, its
            # remaining chains are drained synchronously (emission order IS
            # program order — a read emitted before the write would see
            # garbage).  K^T chains and V chains key on (pair, 0).
            def junk_steps():
                # end-filler: keeps the PE busy through the final (otherwise
                # slack) blocks so HAM doesn't re-throttle right before the
                # output projection.
                ps = psY.tile([128, 512], f32, name="wu_ps", tag="py")
                mm(ps[:], wu[:, 0:128], wu[:], start=True, stop=True)
                yield 512

            pending = deque()
            pending.append(((0, 1), qk_chain_steps(0, 1)))
            for p in range(1, PAIRS):
                pending.append(((p, 0), qk_chain_steps(p, 0)))
                pending.append(((p, 0), qk_chain_steps(PAIRS + p, 0)))
                pending.append(((p, 0), qk_chain_steps(PAIRS + p, 1)))
                pending.append(((p, 1), qk_chain_steps(p, 1)))
                for i in range(NT):
                    pending.append(((p, 0), v_chain_steps(p, i)))
            for _ in range(12):
                pending.append(((-1, -1), junk_steps()))
            active = deque()

            def filler_step(budget_cols):
                while budget_cols > 0:
                    while len(active) < 2 and pending:
                        active.append(pending.popleft())
                    if not active:
                        return
                    dep, g = active.popleft()
                    try:
                        budget_cols -= next(g)
                        active.append((dep, g))
                    except StopIteration:
                        pass

            def drain_block(key):
                for q in (active, pending):
                    keep = [(dep, g) for dep, g in q if dep != key]
                    drain = [g for dep, g in q if dep == key]
                    q.clear()
                    q.extend(keep)
                    for g in drain:
                        run_chain(g)

            # ---- prologue ----
            # Warm-up matmuls on a junk tile while the input DMAs land: HAM
            # un-throttles the PE clock (1.2 -> 2.4 GHz) after ~3.4us of
            # activity, so the real projection chains run at full speed.
            wu = pers.tile([128, 512], f16, name="wu", tag="wu")
            nc.vector.memset(wu[:], 0.0)
            for _ in range(8):
                ps = psY.tile([128, 512], f32, name="wu_ps", tag="py")
                mm(ps[:], wu[:, 0:128], wu[:], start=True, stop=True)
            # pair 0/6 projections + pair-0 V (q j1-half rides the filler)
            run_chain(qk_chain_steps(0, 0))
            run_chain(qk_chain_steps(6, 0))
            run_chain(qk_chain_steps(6, 1))
            for i in range(NT):
                run_chain(v_chain_steps(0, i))

            # ---- attention: software-pipelined over (t, j, i) ----
            iters = [(t, j, i)
                     for t in range(PAIRS) for j in range(2)
                     for i in range(NT)]

            def emit_scores(t, j, i):
                if i == 0:
                    drain_block((t, j))
                s_ps = psS.tile([128, 1024], f32, name="s_ps", tag="ps")
                qt, kt = qkt[t], qkt[PAIRS + t]
                for h in range(2):
                    # S^T[m, n] = sum_d K^T[d, m] Q^T[d, n]; h0/h1 on PE row
                    # groups 0/64 run concurrently.
                    mm(s_ps[:, 512 * h:512 * (h + 1)],
                       kt[64 * h:64 * (h + 1), 128 * i:128 * (i + 1)],
                       qt[64 * h:64 * (h + 1), 512 * j:512 * (j + 1)],
                       start=True, stop=True)
                return s_ps

            s_queue = deque()
            s_queue.append(emit_scores(*iters[0]))
            pv_ps = None
            defer1 = deque()
            defer2 = deque()
            for n, (t, j, i) in enumerate(iters):
                if i == 0:
                    pv_ps = [psY.tile([65, 512], f32, name=f"pv{h}", tag="py")
                             for h in range(2)]
                s_ps = s_queue.popleft()
                stexp = pB.tile([128, 2, 512], f16, name="stexp",
                                tag="stexp", bufs=6)
                # exp(S^T / 8) for both heads, PSUM -> SBUF f16
                nc.scalar.activation(
                    stexp[:, :, :],
                    s_ps[:].rearrange("q (h n) -> q h n", h=2),
                    Exp, scale=SCALE)
                # pipeline: next iteration's scores go ahead of this P@V so
                # the PE never waits for ScalarE head-of-line.
                if n + 1 < len(iters):
                    s_queue.append(emit_scores(*iters[n + 1]))
                # fillers go BEFORE P@V in the in-order PE queue: while P@V
                # waits on the exp semaphore, these independent matmuls run.
                filler_step(1344 if t == 0 else 1152)
                for h in range(2):
                    # rows 0:64 = (P~ @ V)^T, row 64 = denominator
                    mm(pv_ps[h][:],
                       vbuf[i][t][:, 65 * h:65 * (h + 1)],
                       stexp[:, h, :],
                       start=(i == 0), stop=(i == NT - 1))

                if i == 1 and defer1:
                    # previous block's reciprocals + broadcast-bounce DMAs:
                    # by now the den-gather DMA has landed, so the (tiny)
                    # DVE reciprocal never waits in the strict-FIFO DVE
                    # queue (a waiting op there delays psum-releasing copies
                    # and stalls the PE long enough for HAM to re-throttle).
                    defer1.popleft()()
                if i == 4 and defer2:
                    # previous block's normalize multiplies: the broadcast
                    # DMA chain (~3us) is done by now.
                    defer2.popleft()()
                if i == NT - 1:
                    # copies free the P@V banks immediately
                    stages = []
                    for h in range(2):
                        stage = pB.tile([65, 512], f32, name="stage",
                                        tag="stage", bufs=4)
                        nc.vector.tensor_copy(stage[:], pv_ps[h][:])
                        stages.append(stage)
                    dens = []
                    for h in range(2):
                        # [1, 512] DVE reciprocal is FD-bound (~3us); DMA the
                        # denominator row into [128, 4] first where the same
                        # op is ~130ns.
                        den_t = pB.tile([128, 4], f32, name="den_t",
                                        tag="den_t", bufs=4)
                        nc.gpsimd.dma_start(den_t[:], stages[h][64:65, :])
                        dens.append(den_t)
                    cell = {}

                    def norm_recips(dens=dens, cell=cell):
                        rbs = []
                        for h in range(2):
                            nc.vector.reciprocal(dens[h][:], dens[h][:])
                            dr2 = pDr.tile([1, 512], f32, name="dr2",
                                           tag="dr2", bufs=4)
                            nc.gpsimd.dma_start(
                                dr2[:].rearrange("q (a b) -> (q a) b", a=128),
                                dens[h][:])
                            # partition-broadcast of the reciprocal row: SBUF
                            # APs can't have zero partition step, so bounce
                            # via DRAM.
                            rb = pB.tile([64, 512], f32, name="rb", tag="rb",
                                         bufs=4)
                            nc.gpsimd.dma_start(
                                rb[:], dr2[:].to_broadcast((64, 512)))
                            rbs.append(rb)
                        cell["rbs"] = rbs

                    def norm_muls(t=t, j=j, stages=stages, cell=cell):
                        rbs = cell["rbs"]
                        nc.vector.tensor_mul(
                            aot[t][0:64, 512 * j:512 * (j + 1)],
                            stages[0][0:64, :], rbs[0][:])
                        tmp = pB.tile([64, 512], f16, name="tmp1", tag="tmp1")
                        nc.vector.tensor_mul(tmp[:], stages[1][0:64, :],
                                             rbs[1][:])
                        # DVE lanes cannot shift partitions; DMA moves the
                        # odd head into partitions 64:128.
                        nc.gpsimd.dma_start(
                            aot[t][64:128, 512 * j:512 * (j + 1)], tmp[:])

                    defer1.append(norm_recips)
                    defer2.append(norm_muls)

            # last block's normalize chain
            while defer1:
                defer1.popleft()()
            while defer2:
                defer2.popleft()()

            # ---- output projection: y = attn_out^T.T @ w_proj^T + b ----
            # y leaves in [128, 384] halves on alternating queues so no
            # single DMA engine serializes the 3MB output.
            for i in range(NT):
                yt = pB.tile([128, C], f32, name="yt", tag="yt")
                for ci, c0 in enumerate((0, 384)):
                    if (2 * i + ci) % 2 == 0:
                        pp = psS.tile([128, 384], f32, name="pp", tag="ps")
                    else:
                        pp = psY.tile([128, 384], f32, name="pp", tag="py")
                    for k in range(KT):
                        mm(pp[:, 0:384],
                           aot[k][:, 128 * i:128 * (i + 1)],
                           wp[k][:, c0:c0 + 384],
                           start=(k == 0), stop=(k == KT - 1))
                    nc.vector.tensor_add(yt[:, c0:c0 + 384], pp[:, 0:384],
                                         bias_t[:, c0:c0 + 384])
                    for q0 in (0, 192):
                        c = c0 + q0
                        eng = (nc.sync, nc.scalar, nc.gpsimd)[(4 * i + 2 * ci
                                                               + q0 // 192) % 3]
                        eng.dma_start(y_d[128 * i:128 * (i + 1), c:c + 192],
                                      yt[:, c:c + 192])

    nc.compile()
    return nc


def make_in_maps(x, w_qkv, w_proj, b_proj):
    wqkvT = np.ascontiguousarray(
        np.asarray(w_qkv, dtype=np.float32).T).astype(np.float16)
    wprojT = np.ascontiguousarray(
        np.asarray(w_proj, dtype=np.float32).T).astype(np.float16)
    bias_rep = np.ascontiguousarray(
        np.broadcast_to(np.asarray(b_proj, dtype=np.float32), (128, C)))
    x = np.asarray(x, dtype=np.float32)
    return [
        {
            "xT": np.ascontiguousarray(x[b].T).astype(np.float16),
            "wqkvT": wqkvT,
            "wprojT": wprojT,
            "bias_rep": bias_rep,
        }
        for b in range(B)
    ]


def kernel(x, w_qkv, w_proj, b_proj):
    from concourse.bass_utils import run_bass_kernel_spmd

    if "nc" not in _CACHE:
        _CACHE["nc"] = build_program()
    nc = _CACHE["nc"]

    in_maps = make_in_maps(x, w_qkv, w_proj, b_proj)
    res = run_bass_kernel_spmd(nc, in_maps, core_ids=list(range(B)))
    out = np.stack([res.results[b]["y"] for b in range(B)], axis=0)
    return out.astype(np.float32)


# revision 55
# speedup vs baseline: 1.0024x; 1.0024x over previous
"""Multi-head attention (B=8, N=1024, C=768, H=12) on 8 Trainium2 NeuronCores.

Sharding: data-parallel, one batch element per core. Each core computes the
full attention block for its batch: QKV projection, per-head softmax(QK^T/8)V,
and the output projection, entirely on-chip (SBUF/PSUM).

Layout (no on-device transposes):
  - host passes x^T [C, N], w_qkv^T [C, 3C], w_proj^T [C, C] in fp16, bias
    replicated to [128, C] f32.
  - Q, K are produced transposed ([d, n], head-dim on partitions); V in
    natural [n, d] layout by swapping lhsT/rhs.
  - scores are computed transposed (S^T[m, n] = K Q^T) so exp(S^T) feeds the
    P@V matmul as the moving operand; V tiles carry an appended ones-column
    so P@V's 65th output row is the softmax denominator for free.
  - normalization multiplies by a reciprocal row broadcast across partitions
    via a DRAM-bounced DMA (SBUF APs cannot partition-broadcast).

Matmul operands are fp16: the warm PE streams 16-bit moving operands at
1 col/cycle @2.4GHz vs 2 cyc/col for fp32r (HW-measured). PSUM stays f32.
The two K=64 score matmuls per tile run concurrently on PE row groups 0/64.

Schedule (the fp32r version was serialization-bound, not engine-bound):
  - the attention i-loop is software-pipelined: scores for iteration n+1 are
    emitted BEFORE P@V of iteration n, so the in-order PE queue never
    head-of-line-blocks on ScalarE's exp;
  - remaining QKV chains and per-(pair,i) V chains are drip-fed as ~2-matmul
    "filler" steps inside the loop (ScalarE is the bottleneck engine at
    ~1.1us/iter; PE has ~0.35us/iter of slack), instead of block-sized lumps
    that starve ScalarE at pair boundaries;
  - input DMAs are split so the critical slices (x j0-halves, pair-0/6
    weight columns) land first, issued from the Sync + Vector queues in
    parallel, with all bulk (x j1, V/proj weights, remaining QKV columns) on
    the fast-dispatch GPSIMD queue;
  - the softmax-denominator bounce + odd-head repartition DMAs also ride the
    GPSIMD queue;
  - PSUM: psS = 2 x [128,1024] (2 banks each) for the pipelined score tiles,
    psY = 4 x [128,512] (1 bank each): 2 pinned by the P@V accumulators of
    the active pair, 2 rotating among filler/projection chains.
"""

import sys
from collections import deque

import numpy as np

if "/opt/trn_rl_repo" not in sys.path:
    sys.path.insert(0, "/opt/trn_rl_repo")

B = 8
N = 1024
C = 768
H = 12
D = 64
SCALE = D ** -0.5
KT = C // 128           # 6 contraction tiles over channels
NT = N // 128           # 8 token tiles
PAIRS = H // 2          # 6 head pairs

_CACHE = {}


def build_program(**_ignored):
    import concourse.bacc as bacc
    import concourse.mybir as mybir
    import concourse.tile as tile

    f32 = mybir.dt.float32
    f16 = mybir.dt.float16
    Exp = mybir.ActivationFunctionType.Exp

    nc = bacc.Bacc("TRN2", target_bir_lowering=False, debug=False)

    xT_d = nc.dram_tensor("xT", [C, N], f16, kind="ExternalInput")
    wqkvT_d = nc.dram_tensor("wqkvT", [C, 3 * C], f16, kind="ExternalInput")
    wprojT_d = nc.dram_tensor("wprojT", [C, C], f16, kind="ExternalInput")
    bias_d = nc.dram_tensor("bias_rep", [128, C], f32, kind="ExternalInput")
    y_d = nc.dram_tensor("y", [N, C], f32, kind="ExternalOutput")

    mm = nc.tensor.matmul

    with tile.TileContext(nc) as tc:
        with tc.tile_pool(name="pers", bufs=1) as pers, \
             tc.tile_pool(name="cyc", bufs=2) as pB, \
             tc.tile_pool(name="dramb", bufs=2, space="DRAM") as pDr, \
             tc.tile_pool(name="ps_s", bufs=2, space="PSUM") as psS, \
             tc.tile_pool(name="ps_y", bufs=4, space="PSUM") as psY:
            # Q^T,K^T tiles [d, n]: tile m holds heads 2m (parts 0:64) and
            # 2m+1 (parts 64:128); m 0..5 = Q, 6..11 = K.  attn_out^T gets
            # its own tiles (SBUF is not tight in fp16).
            qkt = [pers.tile([128, N], f16, name=f"qkt{m}", tag=f"qkt{m}")
                   for m in range(2 * PAIRS)]
            aot = [pers.tile([128, N], f16, name=f"aot{t}", tag=f"aot{t}")
                   for t in range(PAIRS)]
            # V tiles per (n-tile, pair): [128, 130] = [V_h0 |1| V_h1 |1];
            # ones cols 64/129 feed the denominator row of P@V.  Split per
            # pair so filler writes to pair p never serialize pair p' reads.
            vbuf = [[pers.tile([128, 130], f16, name=f"vb{i}_{p}",
                               tag=f"vb{i}_{p}")
                     for p in range(PAIRS)] for i in range(NT)]
            xt = [pers.tile([128, N], f16, name=f"xt{k}", tag=f"xt{k}")
                  for k in range(KT)]
            wqk = [pers.tile([128, 2 * C], f16, name=f"wqk{k}", tag=f"wqk{k}")
                   for k in range(KT)]
            wv = [pers.tile([128, C], f16, name=f"wv{k}", tag=f"wv{k}")
                  for k in range(KT)]
            wp = [pers.tile([128, C], f16, name=f"wp{k}", tag=f"wp{k}")
                  for k in range(KT)]
            bias_t = pers.tile([128, C], f32, name="bias_t", tag="bias_t")

            # ---- input DMAs: critical slices first, queues in parallel ----
            # sync: x halves (contraction operand of every early chain)
            for k in range(KT):
                nc.sync.dma_start(xt[k][:, 0:512],
                                  xT_d[128 * k:128 * (k + 1), 0:512])
            for k in range(KT):
                nc.sync.dma_start(xt[k][:, 512:1024],
                                  xT_d[128 * k:128 * (k + 1), 512:1024])
            # scalar queue (idle until the first exp): pair-0 V columns
            for k in range(KT):
                nc.scalar.dma_start(wv[k][:, 0:128],
                                    wqkvT_d[128 * k:128 * (k + 1),
                                            2 * C:2 * C + 128])
            # gpsimd: pair-0/6 weight columns first, then all bulk
            for k in range(KT):
                nc.gpsimd.dma_start(wqk[k][:, 0:128],
                                    wqkvT_d[128 * k:128 * (k + 1), 0:128])
            for k in range(KT):
                nc.gpsimd.dma_start(wqk[k][:, 768:896],
                                    wqkvT_d[128 * k:128 * (k + 1), 768:896])
            for k in range(KT):
                nc.gpsimd.dma_start(wv[k][:, 128:768],
                                    wqkvT_d[128 * k:128 * (k + 1),
                                            2 * C + 128:3 * C])
            for k in range(KT):
                nc.gpsimd.dma_start(wqk[k][:, 128:768],
                                    wqkvT_d[128 * k:128 * (k + 1), 128:768])
            for k in range(KT):
                nc.gpsimd.dma_start(wqk[k][:, 896:1536],
                                    wqkvT_d[128 * k:128 * (k + 1), 896:1536])
            for k in range(KT):
                nc.gpsimd.dma_start(wp[k][:],
                                    wprojT_d[128 * k:128 * (k + 1), :])
            nc.gpsimd.dma_start(bias_t[:], bias_d[:])

            for i in range(NT):
                for p in range(PAIRS):
                    ones_ap = vbuf[i][p].rearrange("q (t c) -> q t c",
                                                   c=65)[:, :, 64]
                    nc.vector.memset(ones_ap, 1.0)

            # ---- chain emitters (6-matmul accumulation + DVE finish) ----
            def qk_chain_steps(m, j):
                """Q/K projection chain: qkt[m][:, 512j:512j+512]."""
                ps = psY.tile([128, 512], f32, name="qk_ps", tag="py")
                for k in range(KT):
                    mm(ps[:], wqk[k][:, 128 * m:128 * (m + 1)],
                       xt[k][:, 512 * j:512 * (j + 1)],
                       start=(k == 0), stop=(k == KT - 1))
                    yield 512
                nc.vector.tensor_copy(qkt[m][:, 512 * j:512 * (j + 1)], ps[:])

            def v_chain_steps(p, i):
                """V chain for (pair p, n-tile i) -> vbuf[i][p] data cols."""
                ps = psY.tile([128, 512], f32, name="v_ps", tag="py")
                for k in range(KT):
                    mm(ps[:, 0:128], xt[k][:, 128 * i:128 * (i + 1)],
                       wv[k][:, 128 * p:128 * (p + 1)],
                       start=(k == 0), stop=(k == KT - 1))
                    yield 128
                dst = vbuf[i][p].rearrange("q (t c) -> q t c", c=65)[:, :, 0:64]
                nc.vector.tensor_copy(
                    dst, ps[:, 0:128].rearrange("q (t c) -> q t c", c=64))

            def run_chain(g):
                for _ in g:
                    pass

            # filler queue: drip-feeds chain matmuls into the attention loop.
            # Two chains active at once (two rotating psY slots).  Each chain
            # is tagged with the (pair, j) block whose attention reads its
            # output; before that block's first scores are emitted, its
            # remaining chains are drained synchronously (emission order IS
            # program order — a read emitted before the write would see
            # garbage).  K^T chains and V chains key on (pair, 0).
            def junk_steps():
                # end-filler: keeps the PE busy through the final (otherwise
                # slack) blocks so HAM doesn't re-throttle right before the
                # output projection.
                ps = psY.tile([128, 512], f32, name="wu_ps", tag="py")
                mm(ps[:], wu[:, 0:128], wu[:], start=True, stop=True)
                yield 512

            pending = deque()
            pending.append(((0, 1), qk_chain_steps(0, 1)))
            for p in range(1, PAIRS):
                pending.append(((p, 0), qk_chain_steps(p, 0)))
                pending.append(((p, 0), qk_chain_steps(PAIRS + p, 0)))
                pending.append(((p, 0), qk_chain_steps(PAIRS + p, 1)))
                pending.append(((p, 1), qk_chain_steps(p, 1)))
                for i in range(NT):
                    pending.append(((p, 0), v_chain_steps(p, i)))
            for _ in range(4):
                pending.append(((-1, -1), junk_steps()))
            active = deque()

            def filler_step(budget_cols):
                while budget_cols > 0:
                    while len(active) < 2 and pending:
                        active.append(pending.popleft())
                    if not active:
                        return
                    dep, g = active.popleft()
                    try:
                        budget_cols -= next(g)
                        active.append((dep, g))
                    except StopIteration:
                        pass

            def drain_block(key):
                for q in (active, pending):
                    keep = [(dep, g) for dep, g in q if dep != key]
                    drain = [g for dep, g in q if dep == key]
                    q.clear()
                    q.extend(keep)
                    for g in drain:
                        run_chain(g)

            # ---- prologue ----
            # Warm-up matmuls on a junk tile while the input DMAs land: HAM
            # un-throttles the PE clock (1.2 -> 2.4 GHz) after ~3.4us of
            # activity, so the real projection chains run at full speed.
            wu = pers.tile([128, 512], f16, name="wu", tag="wu")
            nc.vector.memset(wu[:], 0.0)
            for _ in range(8):
                ps = psY.tile([128, 512], f32, name="wu_ps", tag="py")
                mm(ps[:], wu[:, 0:128], wu[:], start=True, stop=True)
            # pair 0/6 projections + pair-0 V (q j1-half rides the filler)
            run_chain(qk_chain_steps(0, 0))
            run_chain(qk_chain_steps(6, 0))
            run_chain(qk_chain_steps(6, 1))
            for i in range(NT):
                run_chain(v_chain_steps(0, i))

            # ---- attention: software-pipelined over (t, j, i) ----
            iters = [(t, j, i)
                     for t in range(PAIRS) for j in range(2)
                     for i in range(NT)]

            def emit_scores(t, j, i):
                if i == 0:
                    drain_block((t, j))
                s_ps = psS.tile([128, 1024], f32, name="s_ps", tag="ps")
                qt, kt = qkt[t], qkt[PAIRS + t]
                for h in range(2):
                    # S^T[m, n] = sum_d K^T[d, m] Q^T[d, n]; h0/h1 on PE row
                    # groups 0/64 run concurrently.
                    mm(s_ps[:, 512 * h:512 * (h + 1)],
                       kt[64 * h:64 * (h + 1), 128 * i:128 * (i + 1)],
                       qt[64 * h:64 * (h + 1), 512 * j:512 * (j + 1)],
                       start=True, stop=True)
                return s_ps

            # ---- output projection emitter (y = attn_out^T.T @ w_proj^T
            # + b); y leaves in [128, 192] quarters on rotating queues so no
            # single DMA engine serializes the 3MB output ----
            yt_tiles = {}
            proj_done = set()

            def emit_proj(i, c0, ci):
                proj_done.add((i, c0))
                if i not in yt_tiles:
                    yt_tiles[i] = pB.tile([128, C], f32, name="yt",
                                          tag="yt", bufs=4)
                yt = yt_tiles[i]
                if (2 * i + ci) % 2 == 0:
                    pp = psS.tile([128, 384], f32, name="pp", tag="ps")
                else:
                    pp = psY.tile([128, 384], f32, name="pp", tag="py")
                for k in range(KT):
                    mm(pp[:, 0:384],
                       aot[k][:, 128 * i:128 * (i + 1)],
                       wp[k][:, c0:c0 + 384],
                       start=(k == 0), stop=(k == KT - 1))
                nc.vector.tensor_add(yt[:, c0:c0 + 384], pp[:, 0:384],
                                     bias_t[:, c0:c0 + 384])
                for q0 in (0, 192):
                    c = c0 + q0
                    eng = (nc.sync, nc.scalar, nc.gpsimd)[(4 * i + 2 * ci
                                                           + q0 // 192) % 3]
                    eng.dma_start(y_d[128 * i:128 * (i + 1), c:c + 192],
                                  yt[:, c:c + 192])

            s_queue = deque()
            s_queue.append(emit_scores(*iters[0]))
            pv_ps = None
            defer1 = deque()
            defer2 = deque()
            for n, (t, j, i) in enumerate(iters):
                if i == 0:
                    pv_ps = [psY.tile([65, 512], f32, name=f"pv{h}", tag="py")
                             for h in range(2)]
                s_ps = s_queue.popleft()
                stexp = pB.tile([128, 2, 512], f16, name="stexp",
                                tag="stexp", bufs=6)
                # exp(S^T / 8) for both heads, PSUM -> SBUF f16
                nc.scalar.activation(
                    stexp[:, :, :],
                    s_ps[:].rearrange("q (h n) -> q h n", h=2),
                    Exp, scale=SCALE)
                # pipeline: next iteration's scores go ahead of this P@V so
                # the PE never waits for ScalarE head-of-line.
                if n + 1 < len(iters):
                    s_queue.append(emit_scores(*iters[n + 1]))
                # fillers go BEFORE P@V in the in-order PE queue: while P@V
                # waits on the exp semaphore, these independent matmuls run.
                filler_step(1344 if t == 0 else 1152)
                if t == PAIRS - 1 and j == 1 and i >= 5:
                    # tail overlap: proj tiles i'<=2 read only j0 halves of
                    # attn_out^T (complete since this block's iter ~5), so
                    # they run here while ScalarE finishes the last exps.
                    ip = i - 5
                    emit_proj(ip, 0, 1)
                    emit_proj(ip, 384, 1)
                for h in range(2):
                    # rows 0:64 = (P~ @ V)^T, row 64 = denominator
                    mm(pv_ps[h][:],
                       vbuf[i][t][:, 65 * h:65 * (h + 1)],
                       stexp[:, h, :],
                       start=(i == 0), stop=(i == NT - 1))

                if i == 1 and defer1:
                    # previous block's reciprocals + broadcast-bounce DMAs:
                    # by now the den-gather DMA has landed, so the (tiny)
                    # DVE reciprocal never waits in the strict-FIFO DVE
                    # queue (a waiting op there delays psum-releasing copies
                    # and stalls the PE long enough for HAM to re-throttle).
                    defer1.popleft()()
                if i == 4 and defer2:
                    # previous block's normalize multiplies: the broadcast
                    # DMA chain (~3us) is done by now.
                    defer2.popleft()()
                if i == NT - 1:
                    # copies free the P@V banks immediately
                    stages = []
                    for h in range(2):
                        stage = pB.tile([65, 512], f32, name="stage",
                                        tag="stage", bufs=4)
                        nc.vector.tensor_copy(stage[:], pv_ps[h][:])
                        stages.append(stage)
                    dens = []
                    for h in range(2):
                        # [1, 512] DVE reciprocal is FD-bound (~3us); DMA the
                        # denominator row into [128, 4] first where the same
                        # op is ~130ns.
                        den_t = pB.tile([128, 4], f32, name="den_t",
                                        tag="den_t", bufs=4)
                        nc.gpsimd.dma_start(den_t[:], stages[h][64:65, :])
                        dens.append(den_t)
                    cell = {}

                    def norm_recips(dens=dens, cell=cell):
                        rbs = []
                        for h in range(2):
                            nc.vector.reciprocal(dens[h][:], dens[h][:])
                            dr2 = pDr.tile([1, 512], f32, name="dr2",
                                           tag="dr2", bufs=4)
                            nc.gpsimd.dma_start(
                                dr2[:].rearrange("q (a b) -> (q a) b", a=128),
                                dens[h][:])
                            # partition-broadcast of the reciprocal row: SBUF
                            # APs can't have zero partition step, so bounce
                            # via DRAM.
                            rb = pB.tile([64, 512], f32, name="rb", tag="rb",
                                         bufs=4)
                            nc.gpsimd.dma_start(
                                rb[:], dr2[:].to_broadcast((64, 512)))
                            rbs.append(rb)
                        cell["rbs"] = rbs

                    def norm_muls(t=t, j=j, stages=stages, cell=cell):
                        rbs = cell["rbs"]
                        nc.vector.tensor_mul(
                            aot[t][0:64, 512 * j:512 * (j + 1)],
                            stages[0][0:64, :], rbs[0][:])
                        tmp = pB.tile([64, 512], f16, name="tmp1", tag="tmp1")
                        nc.vector.tensor_mul(tmp[:], stages[1][0:64, :],
                                             rbs[1][:])
                        # DVE lanes cannot shift partitions; DMA moves the
                        # odd head into partitions 64:128.
                        nc.gpsimd.dma_start(
                            aot[t][64:128, 512 * j:512 * (j + 1)], tmp[:])

                    defer1.append(norm_recips)
                    defer2.append(norm_muls)

            # last block's normalize chain
            while defer1:
                defer1.popleft()()
            while defer2:
                defer2.popleft()()

            # ---- remaining output projection tiles ----
            for i in range(NT):
                for ci, c0 in enumerate((0, 384)):
                    if (i, c0) not in proj_done:
                        emit_proj(i, c0, ci)

    nc.compile()
    return nc


def make_in_maps(x, w_qkv, w_proj, b_proj):
    wqkvT = np.ascontiguousarray(
        np.asarray(w_qkv, dtype=np.float32).T).astype(np.float16)
    wprojT = np.ascontiguousarray(
        np.asarray(w_proj, dtype=np.float32).T).astype(np.float16)
    bias_rep = np.ascontiguousarray(
        np.broadcast_to(np.asarray(b_proj, dtype=np.float32), (128, C)))
    x = np.asarray(x, dtype=np.float32)
    return [
        {
            "xT": np.ascontiguousarray(x[b].T).astype(np.float16),
            "wqkvT": wqkvT,
            "wprojT": wprojT,
            "bias_rep": bias_rep,
        }
        for b in range(B)
    ]


def kernel(x, w_qkv, w_proj, b_proj):
    from concourse.bass_utils import run_bass_kernel_spmd

    if "nc" not in _CACHE:
        _CACHE["nc"] = build_program()
    nc = _CACHE["nc"]

    in_maps = make_in_maps(x, w_qkv, w_proj, b_proj)
    res = run_bass_kernel_spmd(nc, in_maps, core_ids=list(range(B)))
    out = np.stack([res.results[b]["y"] for b in range(B)], axis=0)
    return out.astype(np.float32)


# revision 56
# speedup vs baseline: 1.1001x; 1.0975x over previous
"""Multi-head attention (B=8, N=1024, C=768, H=12) on 8 Trainium2 NeuronCores.

Sharding: data-parallel, one batch element per core. Each core computes the
full attention block for its batch: QKV projection, per-head softmax(QK^T/8)V,
and the output projection, entirely on-chip (SBUF/PSUM).

Layout (no on-device transposes):
  - host passes x^T [C, N], w_qkv^T [C, 3C], w_proj^T [C, C] in fp16, bias
    replicated to [128, C] f32.
  - Q, K are produced transposed ([d, n], head-dim on partitions); V in
    natural [n, d] layout by swapping lhsT/rhs.
  - scores are computed transposed (S^T[m, n] = K Q^T) so exp(S^T) feeds the
    P@V matmul as the moving operand; V tiles carry an appended ones-column
    so P@V's 65th output row is the softmax denominator for free.
  - normalization multiplies by a reciprocal row broadcast across partitions
    via a DRAM-bounced DMA (SBUF APs cannot partition-broadcast).

Matmul operands are fp16: the warm PE streams 16-bit moving operands at
1 col/cycle @2.4GHz vs 2 cyc/col for fp32r (HW-measured). PSUM stays f32.
The two K=64 score matmuls per tile run concurrently on PE row groups 0/64.

Schedule (the fp32r version was serialization-bound, not engine-bound):
  - the attention i-loop is software-pipelined: scores for iteration n+1 are
    emitted BEFORE P@V of iteration n, so the in-order PE queue never
    head-of-line-blocks on ScalarE's exp;
  - remaining QKV chains and per-(pair,i) V chains are drip-fed as ~2-matmul
    "filler" steps inside the loop (ScalarE is the bottleneck engine at
    ~1.1us/iter; PE has ~0.35us/iter of slack), instead of block-sized lumps
    that starve ScalarE at pair boundaries;
  - input DMAs are split so the critical slices (x j0-halves, pair-0/6
    weight columns) land first, issued from the Sync + Vector queues in
    parallel, with all bulk (x j1, V/proj weights, remaining QKV columns) on
    the fast-dispatch GPSIMD queue;
  - the softmax-denominator bounce + odd-head repartition DMAs also ride the
    GPSIMD queue;
  - PSUM: psS = 2 x [128,1024] (2 banks each) for the pipelined score tiles,
    psY = 4 x [128,512] (1 bank each): 2 pinned by the P@V accumulators of
    the active pair, 2 rotating among filler/projection chains.
"""

import sys
from collections import deque

import numpy as np

if "/opt/trn_rl_repo" not in sys.path:
    sys.path.insert(0, "/opt/trn_rl_repo")

B = 8
N = 1024
C = 768
H = 12
D = 64
SCALE = D ** -0.5
KT = C // 128           # 6 contraction tiles over channels
NT = N // 128           # 8 token tiles
PAIRS = H // 2          # 6 head pairs

_CACHE = {}


def build_program(**_ignored):
    import concourse.bacc as bacc
    import concourse.mybir as mybir
    import concourse.tile as tile

    f32 = mybir.dt.float32
    f16 = mybir.dt.float16
    Exp = mybir.ActivationFunctionType.Exp

    nc = bacc.Bacc("TRN2", target_bir_lowering=False, debug=False)

    xT_d = nc.dram_tensor("xT", [C, N], f16, kind="ExternalInput")
    wqkvT_d = nc.dram_tensor("wqkvT", [C, 3 * C], f16, kind="ExternalInput")
    wprojT_d = nc.dram_tensor("wprojT", [C, C], f16, kind="ExternalInput")
    bias_d = nc.dram_tensor("bias_rep", [128, C], f32, kind="ExternalInput")
    y_d = nc.dram_tensor("y", [N, C], f32, kind="ExternalOutput")

    mm = nc.tensor.matmul

    with tile.TileContext(nc) as tc:
        with tc.tile_pool(name="pers", bufs=1) as pers, \
             tc.tile_pool(name="cyc", bufs=2) as pB, \
             tc.tile_pool(name="dramb", bufs=2, space="DRAM") as pDr, \
             tc.tile_pool(name="ps_s", bufs=2, space="PSUM") as psS, \
             tc.tile_pool(name="ps_y", bufs=4, space="PSUM") as psY:
            # Q^T,K^T tiles [d, n]: tile m holds heads 2m (parts 0:64) and
            # 2m+1 (parts 64:128); m 0..5 = Q, 6..11 = K.  attn_out^T gets
            # its own tiles (SBUF is not tight in fp16).
            qkt = [pers.tile([128, N], f16, name=f"qkt{m}", tag=f"qkt{m}")
                   for m in range(2 * PAIRS)]
            aot = [pers.tile([128, N], f16, name=f"aot{t}", tag=f"aot{t}")
                   for t in range(PAIRS)]
            # V tiles per (n-tile, pair): [128, 130] = [V_h0 |1| V_h1 |1];
            # ones cols 64/129 feed the denominator row of P@V.  Split per
            # pair so filler writes to pair p never serialize pair p' reads.
            vbuf = [[pers.tile([128, 130], f16, name=f"vb{i}_{p}",
                               tag=f"vb{i}_{p}")
                     for p in range(PAIRS)] for i in range(NT)]
            xt = [pers.tile([128, N], f16, name=f"xt{k}", tag=f"xt{k}")
                  for k in range(KT)]
            wqk = [pers.tile([128, 2 * C], f16, name=f"wqk{k}", tag=f"wqk{k}")
                   for k in range(KT)]
            wv = [pers.tile([128, C], f16, name=f"wv{k}", tag=f"wv{k}")
                  for k in range(KT)]
            wp = [pers.tile([128, C], f16, name=f"wp{k}", tag=f"wp{k}")
                  for k in range(KT)]
            bias_t = pers.tile([128, C], f32, name="bias_t", tag="bias_t")

            # ---- input DMAs: critical slices first, queues in parallel ----
            # sync: x halves (contraction operand of every early chain)
            for k in range(KT):
                nc.sync.dma_start(xt[k][:, 0:512],
                                  xT_d[128 * k:128 * (k + 1), 0:512])
            for k in range(KT):
                nc.sync.dma_start(xt[k][:, 512:1024],
                                  xT_d[128 * k:128 * (k + 1), 512:1024])
            # scalar queue (idle until the first exp): pair-0 V columns
            for k in range(KT):
                nc.scalar.dma_start(wv[k][:, 0:128],
                                    wqkvT_d[128 * k:128 * (k + 1),
                                            2 * C:2 * C + 128])
            # gpsimd: pair-0/6 weight columns first, then all bulk
            for k in range(KT):
                nc.gpsimd.dma_start(wqk[k][:, 0:128],
                                    wqkvT_d[128 * k:128 * (k + 1), 0:128])
            for k in range(KT):
                nc.gpsimd.dma_start(wqk[k][:, 768:896],
                                    wqkvT_d[128 * k:128 * (k + 1), 768:896])
            for k in range(KT):
                nc.gpsimd.dma_start(wv[k][:, 128:768],
                                    wqkvT_d[128 * k:128 * (k + 1),
                                            2 * C + 128:3 * C])
            for k in range(KT):
                nc.gpsimd.dma_start(wqk[k][:, 128:768],
                                    wqkvT_d[128 * k:128 * (k + 1), 128:768])
            for k in range(KT):
                nc.gpsimd.dma_start(wqk[k][:, 896:1536],
                                    wqkvT_d[128 * k:128 * (k + 1), 896:1536])
            for k in range(KT):
                nc.gpsimd.dma_start(wp[k][:],
                                    wprojT_d[128 * k:128 * (k + 1), :])
            nc.gpsimd.dma_start(bias_t[:], bias_d[:])

            for i in range(NT):
                for p in range(PAIRS):
                    ones_ap = vbuf[i][p].rearrange("q (t c) -> q t c",
                                                   c=65)[:, :, 64]
                    nc.vector.memset(ones_ap, 1.0)

            # ---- chain emitters (6-matmul accumulation + DVE finish) ----
            def qk_chain_steps(m, j):
                """Q/K projection chain: qkt[m][:, 512j:512j+512]."""
                ps = psY.tile([128, 512], f32, name="qk_ps", tag="py")
                for k in range(KT):
                    mm(ps[:], wqk[k][:, 128 * m:128 * (m + 1)],
                       xt[k][:, 512 * j:512 * (j + 1)],
                       start=(k == 0), stop=(k == KT - 1))
                    yield 512
                nc.vector.tensor_copy(qkt[m][:, 512 * j:512 * (j + 1)], ps[:])

            def v_chain_steps(p, i):
                """V chain for (pair p, n-tile i) -> vbuf[i][p] data cols."""
                ps = psY.tile([128, 512], f32, name="v_ps", tag="py")
                for k in range(KT):
                    mm(ps[:, 0:128], xt[k][:, 128 * i:128 * (i + 1)],
                       wv[k][:, 128 * p:128 * (p + 1)],
                       start=(k == 0), stop=(k == KT - 1))
                    yield 128
                dst = vbuf[i][p].rearrange("q (t c) -> q t c", c=65)[:, :, 0:64]
                nc.vector.tensor_copy(
                    dst, ps[:, 0:128].rearrange("q (t c) -> q t c", c=64))

            def run_chain(g):
                for _ in g:
                    pass

            # filler queue: drip-feeds chain matmuls into the attention loop.
            # Two chains active at once (two rotating psY slots).  Each chain
            # is tagged with the (pair, j) block whose attention reads its
            # output; before that block's first scores are emitted, its
            # remaining chains are drained synchronously (emission order IS
            # program order — a read emitted before the write would see
            # garbage).  K^T chains and V chains key on (pair, 0).
            def junk_steps():
                # end-filler: keeps the PE busy through the final (otherwise
                # slack) blocks so HAM doesn't re-throttle right before the
                # output projection.
                ps = psY.tile([128, 512], f32, name="wu_ps", tag="py")
                mm(ps[:], wu[:, 0:128], wu[:], start=True, stop=True)
                yield 512

            pending = deque()
            pending.append(((0, 1), qk_chain_steps(0, 1)))
            for p in range(1, PAIRS):
                pending.append(((p, 0), qk_chain_steps(p, 0)))
                pending.append(((p, 0), qk_chain_steps(PAIRS + p, 0)))
                pending.append(((p, 0), qk_chain_steps(PAIRS + p, 1)))
                pending.append(((p, 1), qk_chain_steps(p, 1)))
                for i in range(NT):
                    pending.append(((p, 0), v_chain_steps(p, i)))
            for _ in range(4):
                pending.append(((-1, -1), junk_steps()))
            active = deque()

            def filler_step(budget_cols):
                while budget_cols > 0:
                    while len(active) < 2 and pending:
                        active.append(pending.popleft())
                    if not active:
                        return
                    dep, g = active.popleft()
                    try:
                        budget_cols -= next(g)
                        active.append((dep, g))
                    except StopIteration:
                        pass

            def drain_block(key):
                for q in (active, pending):
                    keep = [(dep, g) for dep, g in q if dep != key]
                    drain = [g for dep, g in q if dep == key]
                    q.clear()
                    q.extend(keep)
                    for g in drain:
                        run_chain(g)

            # ---- prologue ----
            # Warm-up matmuls on a junk tile while the input DMAs land: HAM
            # un-throttles the PE clock (1.2 -> 2.4 GHz) after ~3.4us of
            # activity, so the real projection chains run at full speed.
            wu = pers.tile([128, 512], f16, name="wu", tag="wu")
            nc.vector.memset(wu[:], 0.0)
            for _ in range(8):
                ps = psY.tile([128, 512], f32, name="wu_ps", tag="py")
                mm(ps[:], wu[:, 0:128], wu[:], start=True, stop=True)
            # pair 0/6 projections + pair-0 V (q j1-half rides the filler).
            # qk(6,1) goes last: it needs the x j1-halves, the last input
            # DMAs to land — the V chains run while those arrive.
            run_chain(qk_chain_steps(0, 0))
            run_chain(qk_chain_steps(6, 0))
            for i in range(NT):
                run_chain(v_chain_steps(0, i))
            run_chain(qk_chain_steps(6, 1))

            # ---- attention: software-pipelined over (t, j, i) ----
            iters = [(t, j, i)
                     for t in range(PAIRS) for j in range(2)
                     for i in range(NT)]

            def emit_scores(t, j, i):
                if i == 0:
                    drain_block((t, j))
                s_ps = psS.tile([128, 1024], f32, name="s_ps", tag="ps")
                qt, kt = qkt[t], qkt[PAIRS + t]
                for h in range(2):
                    # S^T[m, n] = sum_d K^T[d, m] Q^T[d, n]; h0/h1 on PE row
                    # groups 0/64 run concurrently.
                    mm(s_ps[:, 512 * h:512 * (h + 1)],
                       kt[64 * h:64 * (h + 1), 128 * i:128 * (i + 1)],
                       qt[64 * h:64 * (h + 1), 512 * j:512 * (j + 1)],
                       start=True, stop=True)
                return s_ps

            # ---- output projection emitter (y = attn_out^T.T @ w_proj^T
            # + b); y leaves in [128, 192] quarters on rotating queues so no
            # single DMA engine serializes the 3MB output ----
            yt_tiles = {}
            proj_done = set()

            def emit_proj(i, c0, ci):
                proj_done.add((i, c0))
                if i not in yt_tiles:
                    yt_tiles[i] = pB.tile([128, C], f32, name="yt",
                                          tag="yt", bufs=4)
                yt = yt_tiles[i]
                if (2 * i + ci) % 2 == 0:
                    pp = psS.tile([128, 384], f32, name="pp", tag="ps")
                else:
                    pp = psY.tile([128, 384], f32, name="pp", tag="py")
                for k in range(KT):
                    mm(pp[:, 0:384],
                       aot[k][:, 128 * i:128 * (i + 1)],
                       wp[k][:, c0:c0 + 384],
                       start=(k == 0), stop=(k == KT - 1))
                nc.vector.tensor_add(yt[:, c0:c0 + 384], pp[:, 0:384],
                                     bias_t[:, c0:c0 + 384])
                for q0 in (0, 192):
                    c = c0 + q0
                    eng = (nc.sync, nc.scalar, nc.gpsimd)[(4 * i + 2 * ci
                                                           + q0 // 192) % 3]
                    eng.dma_start(y_d[128 * i:128 * (i + 1), c:c + 192],
                                  yt[:, c:c + 192])

            s_queue = deque()
            s_queue.append(emit_scores(*iters[0]))
            pv_ps = None
            defer1 = deque()
            defer2 = deque()
            for n, (t, j, i) in enumerate(iters):
                if i == 0:
                    pv_ps = [psY.tile([65, 512], f32, name=f"pv{h}", tag="py")
                             for h in range(2)]
                s_ps = s_queue.popleft()
                stexp = pB.tile([128, 2, 512], f16, name="stexp",
                                tag="stexp", bufs=6)
                # exp(S^T / 8) for both heads, PSUM -> SBUF f16
                nc.scalar.activation(
                    stexp[:, :, :],
                    s_ps[:].rearrange("q (h n) -> q h n", h=2),
                    Exp, scale=SCALE)
                # pipeline: next iteration's scores go ahead of this P@V so
                # the PE never waits for ScalarE head-of-line.
                if n + 1 < len(iters):
                    s_queue.append(emit_scores(*iters[n + 1]))
                # fillers go BEFORE P@V in the in-order PE queue: while P@V
                # waits on the exp semaphore, these independent matmuls run.
                filler_step(1344 if t == 0 else 1152)
                if t == PAIRS - 1 and j == 1 and i >= 5:
                    # tail overlap: proj tiles i'<=2 read only j0 halves of
                    # attn_out^T (complete since this block's iter ~5), so
                    # they run here while ScalarE finishes the last exps.
                    ip = i - 5
                    emit_proj(ip, 0, 1)
                    emit_proj(ip, 384, 1)
                for h in range(2):
                    # rows 0:64 = (P~ @ V)^T, row 64 = denominator
                    mm(pv_ps[h][:],
                       vbuf[i][t][:, 65 * h:65 * (h + 1)],
                       stexp[:, h, :],
                       start=(i == 0), stop=(i == NT - 1))

                if i == 1 and defer1:
                    # previous block's reciprocals + broadcast-bounce DMAs:
                    # by now the den-gather DMA has landed, so the (tiny)
                    # DVE reciprocal never waits in the strict-FIFO DVE
                    # queue (a waiting op there delays psum-releasing copies
                    # and stalls the PE long enough for HAM to re-throttle).
                    defer1.popleft()()
                if i == 4 and defer2:
                    # previous block's normalize multiplies: the broadcast
                    # DMA chain (~3us) is done by now.
                    defer2.popleft()()
                if i == NT - 1:
                    # copies free the P@V banks immediately
                    stages = []
                    for h in range(2):
                        stage = pB.tile([65, 512], f32, name="stage",
                                        tag="stage", bufs=4)
                        nc.vector.tensor_copy(stage[:], pv_ps[h][:])
                        stages.append(stage)
                    dens = []
                    for h in range(2):
                        # [1, 512] DVE reciprocal is FD-bound (~3us); DMA the
                        # denominator row into [128, 4] first where the same
                        # op is ~130ns.
                        den_t = pB.tile([128, 4], f32, name="den_t",
                                        tag="den_t", bufs=4)
                        nc.gpsimd.dma_start(den_t[:], stages[h][64:65, :])
                        dens.append(den_t)
                    cell = {}

                    def norm_recips(dens=dens, cell=cell):
                        rbs = []
                        for h in range(2):
                            nc.vector.reciprocal(dens[h][:], dens[h][:])
                            dr2 = pDr.tile([1, 512], f32, name="dr2",
                                           tag="dr2", bufs=4)
                            nc.gpsimd.dma_start(
                                dr2[:].rearrange("q (a b) -> (q a) b", a=128),
                                dens[h][:])
                            # partition-broadcast of the reciprocal row: SBUF
                            # APs can't have zero partition step, so bounce
                            # via DRAM.
                            rb = pB.tile([64, 512], f32, name="rb", tag="rb",
                                         bufs=4)
                            nc.gpsimd.dma_start(
                                rb[:], dr2[:].to_broadcast((64, 512)))
                            rbs.append(rb)
                        cell["rbs"] = rbs

                    def norm_muls(t=t, j=j, stages=stages, cell=cell):
                        rbs = cell["rbs"]
                        nc.vector.tensor_mul(
                            aot[t][0:64, 512 * j:512 * (j + 1)],
                            stages[0][0:64, :], rbs[0][:])
                        tmp = pB.tile([64, 512], f16, name="tmp1", tag="tmp1")
                        nc.vector.tensor_mul(tmp[:], stages[1][0:64, :],
                                             rbs[1][:])
                        # DVE lanes cannot shift partitions; DMA moves the
                        # odd head into partitions 64:128.
                        nc.gpsimd.dma_start(
                            aot[t][64:128, 512 * j:512 * (j + 1)], tmp[:])

                    defer1.append(norm_recips)
                    defer2.append(norm_muls)

            # last block's normalize chain
            while defer1:
                defer1.popleft()()
            while defer2:
                defer2.popleft()()

            # ---- remaining output projection tiles ----
            for i in range(NT):
                for ci, c0 in enumerate((0, 384)):
                    if (i, c0) not in proj_done:
                        emit_proj(i, c0, ci)

    nc.compile()
    return nc


def make_in_maps(x, w_qkv, w_proj, b_proj):
    wqkvT = np.ascontiguousarray(
        np.asarray(w_qkv, dtype=np.float32).T).astype(np.float16)
    wprojT = np.ascontiguousarray(
        np.asarray(w_proj, dtype=np.float32).T).astype(np.float16)
    bias_rep = np.ascontiguousarray(
        np.broadcast_to(np.asarray(b_proj, dtype=np.float32), (128, C)))
    x = np.asarray(x, dtype=np.float32)
    return [
        {
            "xT": np.ascontiguousarray(x[b].T).astype(np.float16),
            "wqkvT": wqkvT,
            "wprojT": wprojT,
            "bias_rep": bias_rep,
        }
        for b in range(B)
    ]


def kernel(x, w_qkv, w_proj, b_proj):
    from concourse.bass_utils import run_bass_kernel_spmd

    if "nc" not in _CACHE:
        _CACHE["nc"] = build_program()
    nc = _CACHE["nc"]

    in_maps = make_in_maps(x, w_qkv, w_proj, b_proj)
    res = run_bass_kernel_spmd(nc, in_maps, core_ids=list(range(B)))
    out = np.stack([res.results[b]["y"] for b in range(B)], axis=0)
    return out.astype(np.float32)


# revision 57
# speedup vs baseline: 1.1239x; 1.0216x over previous
"""Multi-head attention (B=8, N=1024, C=768, H=12) on 8 Trainium2 NeuronCores.

Sharding: data-parallel, one batch element per core. Each core computes the
full attention block for its batch: QKV projection, per-head softmax(QK^T/8)V,
and the output projection, entirely on-chip (SBUF/PSUM).

Layout (no on-device transposes):
  - host passes x^T [C, N], w_qkv^T [C, 3C], w_proj^T [C, C] in fp16, bias
    replicated to [128, C] f32.
  - Q, K are produced transposed ([d, n], head-dim on partitions); V in
    natural [n, d] layout by swapping lhsT/rhs.
  - scores are computed transposed (S^T[m, n] = K Q^T) so exp(S^T) feeds the
    P@V matmul as the moving operand; V tiles carry an appended ones-column
    so P@V's 65th output row is the softmax denominator for free.
  - normalization multiplies by a reciprocal row broadcast across partitions
    via a DRAM-bounced DMA (SBUF APs cannot partition-broadcast).

Matmul operands are fp16: the warm PE streams 16-bit moving operands at
1 col/cycle @2.4GHz vs 2 cyc/col for fp32r (HW-measured). PSUM stays f32.
The two K=64 score matmuls per tile run concurrently on PE row groups 0/64.

Schedule (the fp32r version was serialization-bound, not engine-bound):
  - the attention i-loop is software-pipelined: scores for iteration n+1 are
    emitted BEFORE P@V of iteration n, so the in-order PE queue never
    head-of-line-blocks on ScalarE's exp;
  - remaining QKV chains and per-(pair,i) V chains are drip-fed as ~2-matmul
    "filler" steps inside the loop (ScalarE is the bottleneck engine at
    ~1.1us/iter; PE has ~0.35us/iter of slack), instead of block-sized lumps
    that starve ScalarE at pair boundaries;
  - input DMAs are split so the critical slices (x j0-halves, pair-0/6
    weight columns) land first, issued from the Sync + Vector queues in
    parallel, with all bulk (x j1, V/proj weights, remaining QKV columns) on
    the fast-dispatch GPSIMD queue;
  - the softmax-denominator bounce + odd-head repartition DMAs also ride the
    GPSIMD queue;
  - PSUM: psS = 2 x [128,1024] (2 banks each) for the pipelined score tiles,
    psY = 4 x [128,512] (1 bank each): 2 pinned by the P@V accumulators of
    the active pair, 2 rotating among filler/projection chains.
"""

import sys
from collections import deque

import numpy as np

if "/opt/trn_rl_repo" not in sys.path:
    sys.path.insert(0, "/opt/trn_rl_repo")

B = 8
N = 1024
C = 768
H = 12
D = 64
SCALE = D ** -0.5
KT = C // 128           # 6 contraction tiles over channels
NT = N // 128           # 8 token tiles
PAIRS = H // 2          # 6 head pairs

_CACHE = {}


def build_program(**_ignored):
    import concourse.bacc as bacc
    import concourse.mybir as mybir
    import concourse.tile as tile

    f32 = mybir.dt.float32
    f16 = mybir.dt.float16
    Exp = mybir.ActivationFunctionType.Exp

    nc = bacc.Bacc("TRN2", target_bir_lowering=False, debug=False)

    xT_d = nc.dram_tensor("xT", [C, N], f16, kind="ExternalInput")
    wqkvT_d = nc.dram_tensor("wqkvT", [C, 3 * C], f16, kind="ExternalInput")
    wprojT_d = nc.dram_tensor("wprojT", [C, C], f16, kind="ExternalInput")
    bias_d = nc.dram_tensor("bias_rep", [128, C], f32, kind="ExternalInput")
    y_d = nc.dram_tensor("y", [N, C], f32, kind="ExternalOutput")

    mm = nc.tensor.matmul

    with tile.TileContext(nc) as tc:
        with tc.tile_pool(name="pers", bufs=1) as pers, \
             tc.tile_pool(name="cyc", bufs=2) as pB, \
             tc.tile_pool(name="dramb", bufs=2, space="DRAM") as pDr, \
             tc.tile_pool(name="ps_s", bufs=2, space="PSUM") as psS, \
             tc.tile_pool(name="ps_y", bufs=4, space="PSUM") as psY:
            # Q^T,K^T tiles [d, n]: tile m holds heads 2m (parts 0:64) and
            # 2m+1 (parts 64:128); m 0..5 = Q, 6..11 = K.  attn_out^T gets
            # its own tiles (SBUF is not tight in fp16).
            qkt = [pers.tile([128, N], f16, name=f"qkt{m}", tag=f"qkt{m}")
                   for m in range(2 * PAIRS)]
            aot = [pers.tile([128, N], f16, name=f"aot{t}", tag=f"aot{t}")
                   for t in range(PAIRS)]
            # V tiles per (n-tile, pair): [128, 130] = [V_h0 |1| V_h1 |1];
            # ones cols 64/129 feed the denominator row of P@V.  Split per
            # pair so filler writes to pair p never serialize pair p' reads.
            vbuf = [[pers.tile([128, 130], f16, name=f"vb{i}_{p}",
                               tag=f"vb{i}_{p}")
                     for p in range(PAIRS)] for i in range(NT)]
            xt = [pers.tile([128, N], f16, name=f"xt{k}", tag=f"xt{k}")
                  for k in range(KT)]
            wqk = [pers.tile([128, 2 * C], f16, name=f"wqk{k}", tag=f"wqk{k}")
                   for k in range(KT)]
            wv = [pers.tile([128, C], f16, name=f"wv{k}", tag=f"wv{k}")
                  for k in range(KT)]
            wp = [pers.tile([128, C], f16, name=f"wp{k}", tag=f"wp{k}")
                  for k in range(KT)]
            bias_t = pers.tile([128, C], f32, name="bias_t", tag="bias_t")

            # ---- input DMAs: critical slices first, queues in parallel ----
            # sync: x halves (contraction operand of every early chain)
            for k in range(KT):
                nc.sync.dma_start(xt[k][:, 0:512],
                                  xT_d[128 * k:128 * (k + 1), 0:512])
            for k in range(KT):
                nc.sync.dma_start(xt[k][:, 512:1024],
                                  xT_d[128 * k:128 * (k + 1), 512:1024])
            # scalar queue (idle until the first exp): pair-0 V columns
            for k in range(KT):
                nc.scalar.dma_start(wv[k][:, 0:128],
                                    wqkvT_d[128 * k:128 * (k + 1),
                                            2 * C:2 * C + 128])
            # gpsimd: pair-0/6 weight columns first, then all bulk
            for k in range(KT):
                nc.gpsimd.dma_start(wqk[k][:, 0:128],
                                    wqkvT_d[128 * k:128 * (k + 1), 0:128])
            for k in range(KT):
                nc.gpsimd.dma_start(wqk[k][:, 768:896],
                                    wqkvT_d[128 * k:128 * (k + 1), 768:896])
            for k in range(KT):
                nc.gpsimd.dma_start(wv[k][:, 128:768],
                                    wqkvT_d[128 * k:128 * (k + 1),
                                            2 * C + 128:3 * C])
            for k in range(KT):
                nc.gpsimd.dma_start(wqk[k][:, 128:768],
                                    wqkvT_d[128 * k:128 * (k + 1), 128:768])
            for k in range(KT):
                nc.gpsimd.dma_start(wqk[k][:, 896:1536],
                                    wqkvT_d[128 * k:128 * (k + 1), 896:1536])
            for k in range(KT):
                nc.gpsimd.dma_start(wp[k][:],
                                    wprojT_d[128 * k:128 * (k + 1), :])
            nc.gpsimd.dma_start(bias_t[:], bias_d[:])

            for i in range(NT):
                for p in range(PAIRS):
                    ones_ap = vbuf[i][p].rearrange("q (t c) -> q t c",
                                                   c=65)[:, :, 64]
                    nc.vector.memset(ones_ap, 1.0)

            # ---- chain emitters (6-matmul accumulation + DVE finish) ----
            def qk_chain_steps(m, j):
                """Q/K projection chain: qkt[m][:, 512j:512j+512]."""
                ps = psY.tile([128, 512], f32, name="qk_ps", tag="py")
                for k in range(KT):
                    mm(ps[:], wqk[k][:, 128 * m:128 * (m + 1)],
                       xt[k][:, 512 * j:512 * (j + 1)],
                       start=(k == 0), stop=(k == KT - 1))
                    yield 512
                nc.vector.tensor_copy(qkt[m][:, 512 * j:512 * (j + 1)], ps[:])

            def v_chain_steps(p, i):
                """V chain for (pair p, n-tile i) -> vbuf[i][p] data cols."""
                ps = psY.tile([128, 512], f32, name="v_ps", tag="py")
                for k in range(KT):
                    mm(ps[:, 0:128], xt[k][:, 128 * i:128 * (i + 1)],
                       wv[k][:, 128 * p:128 * (p + 1)],
                       start=(k == 0), stop=(k == KT - 1))
                    yield 128
                dst = vbuf[i][p].rearrange("q (t c) -> q t c", c=65)[:, :, 0:64]
                nc.vector.tensor_copy(
                    dst, ps[:, 0:128].rearrange("q (t c) -> q t c", c=64))

            def run_chain(g):
                for _ in g:
                    pass

            # filler queue: drip-feeds chain matmuls into the attention loop.
            # Two chains active at once (two rotating psY slots).  Each chain
            # is tagged with the (pair, j) block whose attention reads its
            # output; before that block's first scores are emitted, its
            # remaining chains are drained synchronously (emission order IS
            # program order — a read emitted before the write would see
            # garbage).  K^T chains and V chains key on (pair, 0).
            def junk_steps():
                # end-filler: keeps the PE busy through the final (otherwise
                # slack) blocks so HAM doesn't re-throttle right before the
                # output projection.
                ps = psY.tile([128, 512], f32, name="wu_ps", tag="py")
                mm(ps[:], wu[:, 0:128], wu[:], start=True, stop=True)
                yield 512

            pending = deque()
            pending.append(((0, 1), qk_chain_steps(0, 1)))
            for p in range(1, PAIRS):
                pending.append(((p, 0), qk_chain_steps(p, 0)))
                pending.append(((p, 0), qk_chain_steps(PAIRS + p, 0)))
                pending.append(((p, 0), qk_chain_steps(PAIRS + p, 1)))
                pending.append(((p, 1), qk_chain_steps(p, 1)))
                for i in range(NT):
                    pending.append(((p, 0), v_chain_steps(p, i)))
            # no junk end-fillers: the early proj chains emitted inside the
            # final block keep the PE warm through the tail
            active = deque()

            def filler_step(budget_cols):
                while budget_cols > 0:
                    while len(active) < 2 and pending:
                        active.append(pending.popleft())
                    if not active:
                        return
                    dep, g = active.popleft()
                    try:
                        budget_cols -= next(g)
                        active.append((dep, g))
                    except StopIteration:
                        pass

            def drain_block(key):
                for q in (active, pending):
                    keep = [(dep, g) for dep, g in q if dep != key]
                    drain = [g for dep, g in q if dep == key]
                    q.clear()
                    q.extend(keep)
                    for g in drain:
                        run_chain(g)

            # ---- prologue ----
            # Warm-up matmuls on a junk tile while the input DMAs land: HAM
            # un-throttles the PE clock (1.2 -> 2.4 GHz) after ~3.4us of
            # activity, so the real projection chains run at full speed.
            wu = pers.tile([128, 512], f16, name="wu", tag="wu")
            nc.vector.memset(wu[:], 0.0)
            for _ in range(8):
                ps = psY.tile([128, 512], f32, name="wu_ps", tag="py")
                mm(ps[:], wu[:, 0:128], wu[:], start=True, stop=True)
            # pair 0/6 projections + pair-0 V (q j1-half rides the filler).
            # qk(6,1) goes last: it needs the x j1-halves, the last input
            # DMAs to land — the V chains run while those arrive.
            run_chain(qk_chain_steps(0, 0))
            run_chain(qk_chain_steps(6, 0))
            for i in range(NT):
                run_chain(v_chain_steps(0, i))
            run_chain(qk_chain_steps(6, 1))

            # ---- attention: software-pipelined over (t, j, i) ----
            iters = [(t, j, i)
                     for t in range(PAIRS) for j in range(2)
                     for i in range(NT)]

            def emit_scores(t, j, i):
                if i == 0:
                    drain_block((t, j))
                s_ps = psS.tile([128, 1024], f32, name="s_ps", tag="ps")
                qt, kt = qkt[t], qkt[PAIRS + t]
                for h in range(2):
                    # S^T[m, n] = sum_d K^T[d, m] Q^T[d, n]; h0/h1 on PE row
                    # groups 0/64 run concurrently.
                    mm(s_ps[:, 512 * h:512 * (h + 1)],
                       kt[64 * h:64 * (h + 1), 128 * i:128 * (i + 1)],
                       qt[64 * h:64 * (h + 1), 512 * j:512 * (j + 1)],
                       start=True, stop=True)
                return s_ps

            # ---- output projection emitter (y = attn_out^T.T @ w_proj^T
            # + b); y leaves in [128, 192] quarters on rotating queues so no
            # single DMA engine serializes the 3MB output ----
            yt_tiles = {}
            proj_done = set()

            def emit_proj(i, c0, ci):
                proj_done.add((i, c0))
                if i not in yt_tiles:
                    yt_tiles[i] = pB.tile([128, C], f32, name="yt",
                                          tag="yt", bufs=4)
                yt = yt_tiles[i]
                if (2 * i + ci) % 2 == 0:
                    pp = psS.tile([128, 384], f32, name="pp", tag="ps")
                else:
                    pp = psY.tile([128, 384], f32, name="pp", tag="py")
                for k in range(KT):
                    mm(pp[:, 0:384],
                       aot[k][:, 128 * i:128 * (i + 1)],
                       wp[k][:, c0:c0 + 384],
                       start=(k == 0), stop=(k == KT - 1))
                nc.vector.tensor_add(yt[:, c0:c0 + 384], pp[:, 0:384],
                                     bias_t[:, c0:c0 + 384])
                for q0 in (0, 192):
                    c = c0 + q0
                    eng = (nc.sync, nc.scalar, nc.gpsimd)[(4 * i + 2 * ci
                                                           + q0 // 192) % 3]
                    eng.dma_start(y_d[128 * i:128 * (i + 1), c:c + 192],
                                  yt[:, c:c + 192])

            s_queue = deque()
            s_queue.append(emit_scores(*iters[0]))
            pv_ps = None
            defer1 = deque()
            defer2 = deque()
            for n, (t, j, i) in enumerate(iters):
                if i == 0:
                    pv_ps = [psY.tile([65, 512], f32, name=f"pv{h}", tag="py")
                             for h in range(2)]
                s_ps = s_queue.popleft()
                stexp = pB.tile([128, 2, 512], f16, name="stexp",
                                tag="stexp", bufs=6)
                # exp(S^T / 8) for both heads, PSUM -> SBUF f16
                nc.scalar.activation(
                    stexp[:, :, :],
                    s_ps[:].rearrange("q (h n) -> q h n", h=2),
                    Exp, scale=SCALE)
                # pipeline: next iteration's scores go ahead of this P@V so
                # the PE never waits for ScalarE head-of-line.
                if n + 1 < len(iters):
                    s_queue.append(emit_scores(*iters[n + 1]))
                # fillers go BEFORE P@V in the in-order PE queue: while P@V
                # waits on the exp semaphore, these independent matmuls run.
                filler_step(1344 if t == 0 else 1152)
                if t == PAIRS - 1 and j == 1 and i >= 5:
                    # tail overlap: proj tiles i'<=2 read only j0 halves of
                    # attn_out^T (complete since this block's iter ~5), so
                    # they run here while ScalarE finishes the last exps.
                    ip = i - 5
                    emit_proj(ip, 0, 1)
                    emit_proj(ip, 384, 1)
                for h in range(2):
                    # rows 0:64 = (P~ @ V)^T, row 64 = denominator
                    mm(pv_ps[h][:],
                       vbuf[i][t][:, 65 * h:65 * (h + 1)],
                       stexp[:, h, :],
                       start=(i == 0), stop=(i == NT - 1))

                if i == 1 and defer1:
                    # previous block's reciprocals + broadcast-bounce DMAs:
                    # by now the den-gather DMA has landed, so the (tiny)
                    # DVE reciprocal never waits in the strict-FIFO DVE
                    # queue (a waiting op there delays psum-releasing copies
                    # and stalls the PE long enough for HAM to re-throttle).
                    defer1.popleft()()
                if i == 4 and defer2:
                    # previous block's normalize multiplies: the broadcast
                    # DMA chain (~3us) is done by now.
                    defer2.popleft()()
                if i == NT - 1:
                    # copies free the P@V banks immediately
                    stages = []
                    for h in range(2):
                        stage = pB.tile([65, 512], f32, name="stage",
                                        tag="stage", bufs=4)
                        nc.vector.tensor_copy(stage[:], pv_ps[h][:])
                        stages.append(stage)
                    dens = []
                    for h in range(2):
                        # [1, 512] DVE reciprocal is FD-bound (~3us); DMA the
                        # denominator row into [128, 4] first where the same
                        # op is ~130ns.
                        den_t = pB.tile([128, 4], f32, name="den_t",
                                        tag="den_t", bufs=4)
                        nc.gpsimd.dma_start(den_t[:], stages[h][64:65, :])
                        dens.append(den_t)
                    cell = {}

                    def norm_recips(dens=dens, cell=cell):
                        rbs = []
                        for h in range(2):
                            nc.vector.reciprocal(dens[h][:], dens[h][:])
                            dr2 = pDr.tile([1, 512], f32, name="dr2",
                                           tag="dr2", bufs=4)
                            nc.gpsimd.dma_start(
                                dr2[:].rearrange("q (a b) -> (q a) b", a=128),
                                dens[h][:])
                            # partition-broadcast of the reciprocal row: SBUF
                            # APs can't have zero partition step, so bounce
                            # via DRAM.
                            rb = pB.tile([64, 512], f32, name="rb", tag="rb",
                                         bufs=4)
                            nc.gpsimd.dma_start(
                                rb[:], dr2[:].to_broadcast((64, 512)))
                            rbs.append(rb)
                        cell["rbs"] = rbs

                    def norm_muls(t=t, j=j, stages=stages, cell=cell):
                        rbs = cell["rbs"]
                        nc.vector.tensor_mul(
                            aot[t][0:64, 512 * j:512 * (j + 1)],
                            stages[0][0:64, :], rbs[0][:])
                        tmp = pB.tile([64, 512], f16, name="tmp1", tag="tmp1")
                        nc.vector.tensor_mul(tmp[:], stages[1][0:64, :],
                                             rbs[1][:])
                        # DVE lanes cannot shift partitions; DMA moves the
                        # odd head into partitions 64:128.
                        nc.gpsimd.dma_start(
                            aot[t][64:128, 512 * j:512 * (j + 1)], tmp[:])

                    defer1.append(norm_recips)
                    defer2.append(norm_muls)

            # last block's normalize chain
            while defer1:
                defer1.popleft()()
            while defer2:
                defer2.popleft()()

            # ---- remaining output projection tiles ----
            for i in range(NT):
                for ci, c0 in enumerate((0, 384)):
                    if (i, c0) not in proj_done:
                        emit_proj(i, c0, ci)

    nc.compile()
    return nc


def make_in_maps(x, w_qkv, w_proj, b_proj):
    wqkvT = np.ascontiguousarray(
        np.asarray(w_qkv, dtype=np.float32).T).astype(np.float16)
    wprojT = np.ascontiguousarray(
        np.asarray(w_proj, dtype=np.float32).T).astype(np.float16)
    bias_rep = np.ascontiguousarray(
        np.broadcast_to(np.asarray(b_proj, dtype=np.float32), (128, C)))
    x = np.asarray(x, dtype=np.float32)
    return [
        {
            "xT": np.ascontiguousarray(x[b].T).astype(np.float16),
            "wqkvT": wqkvT,
            "wprojT": wprojT,
            "bias_rep": bias_rep,
        }
        for b in range(B)
    ]


def kernel(x, w_qkv, w_proj, b_proj):
    from concourse.bass_utils import run_bass_kernel_spmd

    if "nc" not in _CACHE:
        _CACHE["nc"] = build_program()
    nc = _CACHE["nc"]

    in_maps = make_in_maps(x, w_qkv, w_proj, b_proj)
    res = run_bass_kernel_spmd(nc, in_maps, core_ids=list(range(B)))
    out = np.stack([res.results[b]["y"] for b in range(B)], axis=0)
    return out.astype(np.float32)
